# revision 53
# baseline (speedup 1.0000x reference)
"""MLA (multi-head latent attention) Bass kernel for 8 trn2 NeuronCores — v3.

Sharding: core = b*4 + g  (b in {0,1} batches, g in {0..3} head-groups of 4 heads).

Structure (v3): projections and attention are interleaved per query-group so
the tensor engine never idles long enough for the PE HAM clock-gate to drop
to 1.2 GHz:

  for tg in 0..3:
    proj(tg):  q_c / q_rope / k_rope / latent / k_c / V for columns tg
    for h in 0..3: attention stage (h, qg=tg)  [kc pairs, fused exp per pair]
    per-qg epilogue batched across heads (one [128,2048] comb transpose)
    normalization for qg-1 pops one stage behind; outproj(qg-1) streams

Key engine assignments (measured: ~170ns fixed cost per DVE op, PSUM reads
force 1x mode, gpsimd has no PSUM access and no max op):
  tensor: matmuls + additive causal masks (ident @ (-1e4 pattern) accumulated
          into the scores PSUM) + r-broadcast (K=1 ones matmul)
  scalar: exp (one [128,1024] ACT per kc pair) + PSUM->SBUF copies w/ shifts
  vector: row-max tracking (exp'd tiles), emax partition-reduce via 32x32
          transpose + shuffle tree in (j,b)-permuted layout, reciprocal via
          transpose round-trip, remaining PSUM copies
  gpsimd: SBUF-only rope combine ops, big memsets
"""

import math
import os

import numpy as np
import ml_dtypes

import concourse.bass as bass
import concourse.mybir as mybir
import concourse.tile as _tile_mod
from concourse.tile import TileContext
from concourse.vector_clock import ScopedClock, VectorClock
import bass_rust as _bass_rust
from concourse.bass_utils import run_bass_kernel_spmd

_N_PROCS = _bass_rust.N_PROCS
BF = ml_dtypes.bfloat16


def _split_drain_and_barrier(self, tick_clock, wait_clock):
    """Replacement for TileContext._drain_and_barrier: the stock version puts
    the whole global vector clock (up to 27 sem waits) on one Drain, which this
    walrus rejects ("Too many sync wait commands").  Emit one Drain per
    outstanding processor instead."""
    gc = tick_clock.global_clock
    procs = [p for p in range(_N_PROCS) if gc[p] > 0]
    for p in procs:
        vc = VectorClock([gc[q] if q == p else 0 for q in range(_N_PROCS)])
        d = self.nc.sync.drain()
        wait_clock.add_sem_waits(d.ins, ScopedClock({None: vc}))
    self.nc.all_engine_barrier()
    popped = self.nc._tile_sem_poison_stack.pop()
    assert popped is self._sem_poison
    self.nc.clear_and_free_semaphores(list(self.sems.allocated().values()))
    self.nc.all_engine_barrier()


_tile_mod.TileContext._drain_and_barrier = _split_drain_and_barrier

# ---------------------------------------------------------------------------
# This walrus build allows only ONE sync-wait per instruction ("Too many sync
# wait commands").  Post-process the BIR JSON: excess waits are hoisted onto
# same-engine NoOp carriers inserted immediately before the instruction (same
# program point on the engine's sequential stream -> semantics unchanged).
# NoOp (unlike Drain) does not flush the engine pipeline: ~13ns vs ~500ns.
# ---------------------------------------------------------------------------
_orig_to_json_bytes = bass.Bass.to_json_bytes
_WAIT_LIMITS = {"Drain": 1, "DMACopy": 1}
_DEF_WAIT_LIMIT = 1


def _to_json_split_waits(self, *a, **kw):
    import json as _json
    data = _json.loads(_orig_to_json_bytes(self, *a, **kw))
    nid = 0
    for f in data.get("functions", []):
        for bb in f.get("blocks", []):
            out = []
            for inst in bb.get("instructions", []):
                si = inst.get("sync_info")
                if isinstance(si, dict):
                    w = si.get("on_wait")
                    if isinstance(w, list):
                        k = _WAIT_LIMITS.get(inst.get("opcode"), _DEF_WAIT_LIMIT)
                        if len(w) > k:
                            extra, keep = w[:-k], w[-k:]
                            for wt in extra:
                                out.append({
                                    "debug": inst.get("debug"),
                                    "engine": inst["engine"],
                                    "ins": [], "outs": [],
                                    "name": f"wsplit-{nid}",
                                    "opcode": "NoOp",
                                    "sync_info": {"on_update": [],
                                                  "on_wait": [wt]},
                                })
                                nid += 1
                            si["on_wait"] = keep
                out.append(inst)
            bb["instructions"] = out
    return _json.dumps(data).encode()


bass.Bass.to_json_bytes = _to_json_split_waits

B, T, E = 2, 2048, 1024
H, DH = 16, 64
DKV = 256
DR = 32
HL = 4              # heads per core
NG = 4              # head groups
SCALE = 1.0 / math.sqrt(DH + DR)
TG = 512            # query-group width
KC = 128            # key-chunk width
NTG = T // TG       # 4
NKC = T // KC       # 16
EC = E // 128       # 8  e-chunks
CC = DKV // 128     # 2  latent chunks

F32 = mybir.dt.float32
BF16 = mybir.dt.bfloat16
AF = mybir.ActivationFunctionType
ALU = mybir.AluOpType
AX = mybir.AxisListType

_CACHE = {}

_ROT16 = [(i + 16) % 32 for i in range(32)]
_IDENT = list(range(32))


def _build_program():
    nc = bass.Bass()

    xT = nc.declare_dram_parameter("xT", [E, T], BF16, isOutput=False)
    wq = nc.declare_dram_parameter("wq", [E, HL * 128], BF16, isOutput=False)
    wkvd = nc.declare_dram_parameter("wkvd", [E, DKV], BF16, isOutput=False)
    wku = nc.declare_dram_parameter("wku", [DKV, HL * DH], BF16, isOutput=False)
    wvu = nc.declare_dram_parameter("wvu", [DKV, HL * DH], BF16, isOutput=False)
    wo = nc.declare_dram_parameter("wo", [HL * DH, E], BF16, isOutput=False)
    cosq = nc.declare_dram_parameter("cosq", [128, T], BF16, isOutput=False)
    sinq = nc.declare_dram_parameter("sinq", [128, T], BF16, isOutput=False)
    cbc = nc.declare_dram_parameter("cbc", [32, HL * 16], F32, isOutput=False)
    masks = nc.declare_dram_parameter("masks", [128, 4 * TG], BF16, isOutput=False)
    ident = nc.declare_dram_parameter("ident", [128, 128], BF16, isOutput=False)
    out = nc.declare_dram_parameter("out", [T, E], BF16, isOutput=True)

    xTr = xT.rearrange("(c p) t -> p c t", p=128)

    with TileContext(nc) as tc:
        from contextlib import ExitStack

        with ExitStack() as ctx:
            singles = ctx.enter_context(tc.tile_pool(name="singles", bufs=1))
            pool = ctx.enter_context(tc.tile_pool(name="pool", bufs=2))
            psp = ctx.enter_context(tc.tile_pool(name="psp", bufs=1, space="PSUM"))

            # -------- weights + tables (bf16 from host), first-use order ---
            # per-ec chunked loads so the first projection matmul starts early
            wq_sb = singles.tile([128, EC, HL * 128], BF16)
            wqr_ = wq.rearrange("(c p) f -> p c f", p=128)
            for ec_ in range(EC):
                nc.sync.dma_start(out=wq_sb[:, ec_, :], in_=wqr_[:, ec_, :])

            xt_tiles = {}

            def load_xt(tg):
                ts_ = slice(tg * TG, (tg + 1) * TG)
                for ec_ in range(EC):
                    t_ = pool.tile([128, TG], BF16, name=f"xt{tg}_{ec_}",
                                   tag="xt", bufs=16)
                    nc.sync.dma_start(out=t_, in_=xTr[:, ec_, ts_])
                    xt_tiles[(tg, ec_)] = t_

            load_xt(0)
            wkvd_sb = singles.tile([128, EC, DKV], BF16)
            nc.sync.dma_start(
                out=wkvd_sb, in_=wkvd.rearrange("(c p) f -> p c f", p=128))
            wku_sb = singles.tile([128, CC, HL * DH], BF16)
            nc.sync.dma_start(out=wku_sb, in_=wku.rearrange("(c p) f -> p c f", p=128))
            wvu_sb = singles.tile([128, CC, HL * DH], BF16)
            nc.sync.dma_start(out=wvu_sb, in_=wvu.rearrange("(c p) f -> p c f", p=128))
            cos_sb = singles.tile([128, T], BF16)
            nc.sync.dma_start(out=cos_sb, in_=cosq[:, :])
            sin_sb = singles.tile([128, T], BF16)
            nc.sync.dma_start(out=sin_sb, in_=sinq[:, :])
            masks_sb = singles.tile([128, 4, TG], BF16)
            nc.sync.dma_start(
                out=masks_sb, in_=masks.rearrange("p (j y) -> p j y", j=4))
            ident_sb = singles.tile([128, 128], BF16)
            nc.sync.dma_start(out=ident_sb, in_=ident[:, :])
            cbc_sb = singles.tile([32, HL * 16], F32)
            nc.sync.dma_start(out=cbc_sb, in_=cbc[:, :])
            load_xt(1)
            wo_sb = singles.tile([128, 2, E], BF16)
            nc.sync.dma_start(out=wo_sb, in_=wo.rearrange("(c p) e -> p c e", p=128))

            ones1 = singles.tile([1, DH], BF16)
            nc.vector.memset(ones1, 1.0)
            dext = singles.tile([32, HL * TG], F32)
            nc.gpsimd.memset(dext, 1.0)
            rstage = singles.tile([32, HL * TG], BF16)
            nc.gpsimd.memset(rstage, 1.0)

            # ---------------- persistent activation tiles ----------------
            latT_sb = singles.tile([128, CC, T], BF16)
            qT = [singles.tile([96, T], BF16, name=f"qT{h}") for h in range(HL)]
            kT = [singles.tile([96, T], BF16, name=f"kT{h}") for h in range(HL)]
            v_sb = singles.tile([128, NKC, HL, DH + 1], BF16)
            nc.gpsimd.memset(v_sb, 1.0)
            yraw_sb = singles.tile([DH, HL, T], BF16)
            yT_sb = singles.tile([128, 2, T], BF16)

            norm_queue = []

            def drain_norms(n, keep=0):
                for _ in range(n):
                    if len(norm_queue) > keep:
                        norm_queue.pop(0)()

            def emit_norm(h, qg, rback_ap):
                qs = slice(qg * TG, (qg + 1) * TG)
                bcp = psp.tile([DH, TG], F32, name="bc", tag="B", bufs=2)
                nc.tensor.matmul(bcp, ones1, rback_ap)
                nc.vector.tensor_mul(
                    yT_sb[(h % 2) * DH:(h % 2 + 1) * DH, h // 2, qs],
                    yraw_sb[0:DH, h, qs], bcp)

            def emit_outproj(qg):
                for tt in range(4 * qg, 4 * qg + 4):
                    op = psp.tile([128, E], F32, name="op", tag="S", bufs=2)
                    for eg in range(2):
                        for fc in range(2):
                            nc.tensor.matmul(
                                op[:, eg * TG:(eg + 1) * TG],
                                yT_sb[:, fc, tt * 128:(tt + 1) * 128],
                                wo_sb[:, fc, eg * TG:(eg + 1) * TG],
                                start=(fc == 0), stop=(fc == 1))
                    ost = pool.tile([128, E], BF16, name="ost", tag="ost", bufs=2)
                    if tt % 2 == 0:
                        nc.vector.tensor_copy(ost, op)
                    else:
                        nc.scalar.copy(ost, op)
                    nc.sync.dma_start(
                        out=out[tt * 128:(tt + 1) * 128, :], in_=ost)

            def make_proj_fillers(tg):
                """Projection work for query-group tg as ~10 small closures,
                emitted between attention kc-pairs of the previous group so
                the tensor queue always has scores pending for the scalar
                exp pipeline."""
                ts = slice(tg * TG, (tg + 1) * TG)
                st = {}

                def mm8(tile_name, blk, hv):
                    xts = [xt_tiles[(tg, ec)] for ec in range(EC)]
                    if tile_name not in st:
                        st[tile_name] = psp.tile(
                            [128, 2 * TG], F32, name=tile_name, tag="S", bufs=2)
                    sp = st[tile_name]
                    fs = slice(blk * 128, (blk + 1) * 128)
                    for ec in range(EC):
                        nc.tensor.matmul(
                            sp[:, hv * TG:(hv + 1) * TG],
                            wq_sb[:, ec, fs] if tile_name != "lat"
                            else wkvd_sb[:, ec, blk * 128:(blk + 1) * 128],
                            xts[ec],
                            start=(ec == 0), stop=(ec == EC - 1))

                def f0():
                    if tg + 2 < NTG:
                        load_xt(tg + 2)
                    mm8("qc", 0, 0)

                def f1():
                    mm8("qc", 1, 1)
                    qcp = st["qc"]
                    nc.vector.tensor_copy(qT[0][0:DH, ts], qcp[0:DH, 0:TG])
                    nc.scalar.copy(qT[1][0:DH, ts], qcp[DH:128, 0:TG])
                    nc.vector.tensor_copy(qT[2][0:DH, ts], qcp[0:DH, TG:2 * TG])
                    nc.scalar.copy(qT[3][0:DH, ts], qcp[DH:128, TG:2 * TG])

                def f2():
                    mm8("qr", 2, 0)

                def f3():
                    mm8("qr", 3, 1)
                    qrp = st["qr"]
                    # q rope, 4 heads batched; SBUF-only combines on gpsimd
                    sw = pool.tile([128, TG], F32, name="sw", tag="qsw", bufs=2)
                    nc.vector.stream_shuffle(sw, qrp[:, 0:TG], _ROT16)
                    t1 = pool.tile([128, TG], BF16, name="t1", tag="t1", bufs=2)
                    nc.vector.tensor_mul(t1, qrp[:, 0:TG], cos_sb[:, ts])
                    t2 = pool.tile([128, TG], BF16, name="t2", tag="t2", bufs=2)
                    nc.gpsimd.tensor_mul(t2, sw, sin_sb[:, ts])
                    rq = pool.tile([128, TG], BF16, name="rq", tag="rq", bufs=2)
                    nc.gpsimd.tensor_add(rq, t1, t2)
                    for h in range(HL):
                        nc.scalar.copy(qT[h][DH:96, ts],
                                       rq[h * DR:(h + 1) * DR, :])
                    # k rope (rows 0:32 of the second half)
                    sw2 = pool.tile([DR, TG], F32, name="sw2", tag="sw2", bufs=2)
                    nc.vector.stream_shuffle(sw2, qrp[0:DR, TG:2 * TG], _ROT16)
                    k1 = pool.tile([DR, TG], BF16, name="k1", tag="k1", bufs=2)
                    nc.vector.tensor_mul(k1, qrp[0:DR, TG:2 * TG],
                                         cos_sb[0:DR, ts])
                    k2 = pool.tile([DR, TG], BF16, name="k2", tag="k2", bufs=2)
                    nc.gpsimd.tensor_mul(k2, sw2, sin_sb[0:DR, ts])
                    nc.vector.tensor_add(kT[0][DH:96, ts], k1, k2)
                    for h in range(1, HL):
                        nc.scalar.copy(kT[h][DH:96, ts], kT[0][DH:96, ts])

                def f4():
                    mm8("lat", 0, 0)

                def f5():
                    mm8("lat", 1, 1)
                    lp = st["lat"]
                    nc.vector.tensor_copy(latT_sb[:, 0, ts], lp[:, 0:TG])
                    nc.vector.tensor_copy(latT_sb[:, 1, ts], lp[:, TG:2 * TG])

                def kc_f(hp):
                    def f():
                        mp = psp.tile([128, TG], F32, name="kc", tag="M", bufs=2)
                        for cc in range(CC):
                            nc.tensor.matmul(
                                mp, wku_sb[:, cc, hp * 128:(hp + 1) * 128],
                                latT_sb[:, cc, ts],
                                start=(cc == 0), stop=(cc == CC - 1))
                        nc.vector.tensor_copy(kT[2 * hp][0:DH, ts], mp[0:DH, :])
                        nc.scalar.copy(kT[2 * hp + 1][0:DH, ts], mp[DH:128, :])
                    return f

                def v_f(tt0):
                    def f():
                        for tt in (tt0, tt0 + 1):
                            bp = psp.tile([128, HL * DH], F32, name="v",
                                          tag="B", bufs=2)
                            for cc in range(CC):
                                nc.tensor.matmul(
                                    bp, latT_sb[:, cc,
                                                tt * 128:(tt + 1) * 128],
                                    wvu_sb[:, cc, :],
                                    start=(cc == 0), stop=(cc == CC - 1))
                            nc.vector.tensor_copy(v_sb[:, tt, :, 0:DH], bp)
                    return f

                return [f0, f1, f2, f3, f4, f5,
                        kc_f(0), kc_f(1), v_f(4 * tg), v_f(4 * tg + 2)]

            def epilogue_half(qg, hf, comb, nh=2):
                """emax + denominator + reciprocal for heads h0..h0+nh."""
                h0 = 2 * hf if nh == 2 else hf
                cs = slice(h0 * TG, (h0 + nh) * TG)
                bs = slice(h0 * 16, (h0 + nh) * 16)
                W = nh * TG
                combT = pool.tile([128, W], BF16, name="combT", tag=f"combT{W}",
                                  bufs=1)
                nc.vector.transpose(
                    combT, comb.rearrange("p h y -> p (h y)")[:, cs])
                red = pool.tile([128, W // 32], F32, name="red", tag=f"red{W}", bufs=2)
                nc.vector.reduce_max(
                    red, combT.rearrange("p (b j) -> p b j", j=32), axis=AX.X)
                s1 = pool.tile([32, W // 32], F32, name="s1", tag=f"s1{W}", bufs=2)
                nc.vector.stream_shuffle(s1, red[32:64, :], _IDENT)
                s2 = pool.tile([32, W // 32], F32, name="s2", tag=f"s2{W}", bufs=2)
                nc.vector.stream_shuffle(s2, red[64:96, :], _IDENT)
                s3 = pool.tile([32, W // 32], F32, name="s3", tag=f"s3{W}", bufs=2)
                nc.vector.stream_shuffle(s3, red[96:128, :], _IDENT)
                nc.vector.tensor_max(s1, red[0:32, :], s1)
                nc.vector.tensor_max(s2, s2, s3)
                emfP = pool.tile([32, W // 32], F32, name="emf", tag=f"emf{W}", bufs=2)
                nc.vector.tensor_max(emfP, s1, s2)
                # d = dsum + C_h*emax (perm layout); C broadcast from host tile
                nc.vector.tensor_mul(emfP, emfP, cbc_sb[:, bs])
                dTt = pool.tile([32, W], F32, name="dTt", tag=f"dTt{W}", bufs=1)
                nc.vector.transpose(dTt, dext[:, cs])
                dP = pool.tile([32, W // 32], F32, name="dP", tag=f"dP{W}", bufs=2)
                nc.vector.tensor_add(
                    dP, emfP,
                    dTt.rearrange("p (b j) -> p b j", j=32)[:, :, 0])
                with nc.allow_low_precision(reason="r in bf16; tol 2e-2"):
                    nc.vector.reciprocal(
                        rstage[:, cs].rearrange("p (b j) -> p b j", j=32)
                        [:, :, 0], dP)
                rback = pool.tile([32, W], BF16, name="rbk", tag=f"rbk{W}", bufs=2)
                nc.vector.transpose(rback, rstage[:, cs])
                for hh in range(h0, h0 + nh):
                    norm_queue.append(
                        (lambda h_=hh, qq=qg,
                         r_=rback[0:1, (hh - h0) * TG:(hh - h0 + 1) * TG]:
                         emit_norm(h_, qq, r_)))

            # =======================================================
            for f in make_proj_fillers(0):
                f()
            for tg in range(NTG):
                qg = tg
                qs = slice(qg * TG, (qg + 1) * TG)
                nkc = 4 * qg + 4
                npair = nkc // 2
                diag = 4 * qg
                fillers = (make_proj_fillers(tg + 1)
                           if tg + 1 < NTG else [])
                fillers = list(fillers)

                def pop_filler():
                    if fillers:
                        fillers.pop(0)()

                # per-qg comb, all 4 heads side by side
                comb = pool.tile([128, HL, TG], BF16, name="comb", tag="comb",
                                 bufs=2)
                for h in range(HL):
                    yps = psp.tile([DH + 1, TG], F32, name="py", tag="M", bufs=2)

                    def emit_scores(pr):
                        kca, kcb = 2 * pr, 2 * pr + 1
                        sp = psp.tile([128, 2 * TG], F32, name="ps", tag="S",
                                      bufs=3)
                        for hv, kc in ((0, kca), (1, kcb)):
                            half = sp[:, hv * TG:(hv + 1) * TG]
                            j = kc - diag
                            nc.tensor.matmul(
                                half, kT[h][:, kc * KC:(kc + 1) * KC],
                                qT[h][:, qs],
                                start=True, stop=(j < 0))
                            if j >= 0:
                                # additive causal mask via ident matmul
                                w = 128 * j + 128
                                nc.tensor.matmul(
                                    sp[:, hv * TG:hv * TG + w],
                                    ident_sb, masks_sb[:, j, 0:w],
                                    start=False, stop=True)
                        pt = pool.tile([128, 2 * TG], BF16, name="pt", tag="pt",
                                       bufs=4)
                        nc.scalar.activation(pt, sp, AF.Exp)
                        return pt

                    def emit_pv(pr, pt):
                        if pr == 0:
                            nc.vector.tensor_max(
                                comb[:, h, :], pt[:, 0:TG], pt[:, TG:2 * TG])
                        else:
                            nc.vector.tensor_max(
                                comb[:, h, :], comb[:, h, :], pt[:, 0:TG])
                            nc.vector.tensor_max(
                                comb[:, h, :], comb[:, h, :], pt[:, TG:2 * TG])
                        nc.tensor.matmul(
                            yps, v_sb[:, 2 * pr, h, :], pt[:, 0:TG],
                            start=(pr == 0), stop=False)
                        nc.tensor.matmul(
                            yps, v_sb[:, 2 * pr + 1, h, :], pt[:, TG:2 * TG],
                            start=False, stop=(pr == npair - 1))

                    # software-pipeline scores one pair ahead of PV; proj
                    # fillers for the next query-group slot in between pairs
                    prev = emit_scores(0)
                    for pr in range(1, npair):
                        cur = emit_scores(pr)
                        emit_pv(pr - 1, prev)
                        pop_filler()
                        prev = cur
                    emit_pv(npair - 1, prev)
                    pop_filler()
                    nc.vector.tensor_copy(yraw_sb[:, h, qs], yps[0:DH, :])
                    # dsum (f32) into the h-major d row
                    nc.scalar.copy(dext[0:1, h * TG:(h + 1) * TG],
                                   yps[DH:DH + 1, :])
                    if h == 1:
                        epilogue_half(qg, 0, comb)
                    if h >= 2:
                        epilogue_half(qg, h, comb, nh=1)
                    drain_norms(2 if tg == NTG - 1 else 1,
                                keep=0 if tg == NTG - 1 else 2)
                for f in fillers:
                    f()
                norm_queue.append(lambda qq=qg: emit_outproj(qq))
                drain_norms(1, keep=2)
            # tail
            drain_norms(len(norm_queue))

    return nc


def _masks():
    """Additive causal masks: -1e4 on masked (k > q) cells, 0 elsewhere."""
    x = np.arange(128)[:, None]
    y = np.arange(TG)[None, :]
    ms = [np.where(x - y + 128 * j <= 0, 0.0, -1e4).astype(np.float32)
          for j in range(4)]
    return np.concatenate(ms, axis=1)  # [128, 4*TG]


def _rope_tables():
    half = DR // 2
    inv = 1.0 / (10000.0 ** (np.arange(half, dtype=np.float64) / half))
    ang = np.arange(T, dtype=np.float64)[:, None] * inv[None, :]  # (T, half)
    cos = np.cos(ang).T  # (half, T)
    sin = np.sin(ang).T
    cosk = np.concatenate([cos, cos], axis=0)                 # (32, T)
    sink = np.concatenate([-sin, sin], axis=0)
    cosq = np.tile(cosk, (HL, 1))                             # (128, T)
    sinq = np.tile(sink, (HL, 1))
    return cosq.astype(np.float32), sinq.astype(np.float32)


def _bf(x):
    return np.ascontiguousarray(np.asarray(x, dtype=np.float32).astype(BF))


def kernel(x, Wq, Wqr, Wkr, Wkvd, Wku, Wvu, Wo, lobo_log):
    x = np.asarray(x, dtype=np.float32)
    Wq = np.asarray(Wq, dtype=np.float32)
    Wqr = np.asarray(Wqr, dtype=np.float32)
    Wkr = np.asarray(Wkr, dtype=np.float32)
    Wkvd = np.asarray(Wkvd, dtype=np.float32)
    Wku = np.asarray(Wku, dtype=np.float32)
    Wvu = np.asarray(Wvu, dtype=np.float32)
    Wo = np.asarray(Wo, dtype=np.float32)
    lobo_log = np.asarray(lobo_log, dtype=np.float32)

    if "nc" not in _CACHE:
        _CACHE["nc"] = _build_program()
    nc = _CACHE["nc"]

    cosq, sinq = _rope_tables()
    msk = _masks()
    in_maps = []
    for core in range(8):
        b, g = core // NG, core % NG
        hs = slice(g * HL * DH, (g + 1) * HL * DH)
        # packed q-projection weight blocks (128 cols each):
        #   0: [Wq_h0|Wq_h1]*S   1: [Wq_h2|Wq_h3]*S
        #   2: [Wqr_h0..h3]*S    3: [Wkr | zeros]
        wq_pack = np.zeros((E, HL * 128), dtype=np.float32)
        for h in range(HL):
            gh = g * HL + h
            blk, off = h // 2, (h % 2) * DH
            wq_pack[:, blk * 128 + off:blk * 128 + off + DH] = (
                Wq[:, gh * DH:(gh + 1) * DH] * SCALE)
            wq_pack[:, 256 + h * DR:256 + (h + 1) * DR] = (
                Wqr[:, gh * DR:(gh + 1) * DR] * SCALE)
        wq_pack[:, 384:384 + DR] = Wkr
        # C_h broadcast tile matching the (j, h*16+b) permuted layout
        cv = np.exp(lobo_log[g * HL:(g + 1) * HL])
        cbcv = np.broadcast_to(np.repeat(cv, 16)[None, :], (32, HL * 16))
        in_maps.append({
            "xT": _bf(x[b].T),
            "wq": _bf(wq_pack),
            "wkvd": _bf(Wkvd),
            "wku": _bf(Wku[:, hs]),
            "wvu": _bf(Wvu[:, hs]),
            "wo": _bf(Wo[hs, :]),
            "cosq": _bf(cosq), "sinq": _bf(sinq),
            "cbc": np.ascontiguousarray(cbcv, dtype=np.float32),
            "masks": _bf(msk),
            "ident": _bf(np.eye(128, dtype=np.float32)),
        })

    trace = bool(os.environ.get("BASS_TRACE_KERNEL"))
    bkr = run_bass_kernel_spmd(
        nc, in_maps, core_ids=list(range(8)), trace=trace)
    if trace:
        print(f"HW exec time: {bkr.exec_time_ns} ns")
        if bkr.instructions_and_trace is not None:
            print("trace:", bkr.instructions_and_trace[1])
        _CACHE["last_result"] = bkr
    res = bkr.results
    out = np.zeros((B, T, E), dtype=np.float32)
    for core in range(8):
        out[core // NG] += res[core]["out"].astype(np.float32)
    return out


# revision 55
# speedup vs baseline: 1.0429x; 1.0429x over previous
"""MLA (multi-head latent attention) Bass kernel for 8 trn2 NeuronCores — v3.

Sharding: core = b*4 + g  (b in {0,1} batches, g in {0..3} head-groups of 4 heads).

Structure (v3): projections and attention are interleaved per query-group so
the tensor engine never idles long enough for the PE HAM clock-gate to drop
to 1.2 GHz:

  for tg in 0..3:
    proj(tg):  q_c / q_rope / k_rope / latent / k_c / V for columns tg
    for h in 0..3: attention stage (h, qg=tg)  [kc pairs, fused exp per pair]
    per-qg epilogue batched across heads (one [128,2048] comb transpose)
    normalization for qg-1 pops one stage behind; outproj(qg-1) streams

Key engine assignments (measured: ~170ns fixed cost per DVE op, PSUM reads
force 1x mode, gpsimd has no PSUM access and no max op):
  tensor: matmuls + additive causal masks (ident @ (-1e4 pattern) accumulated
          into the scores PSUM) + r-broadcast (K=1 ones matmul)
  scalar: exp (one [128,1024] ACT per kc pair) + PSUM->SBUF copies w/ shifts
  vector: row-max tracking (exp'd tiles), emax partition-reduce via 32x32
          transpose + shuffle tree in (j,b)-permuted layout, reciprocal via
          transpose round-trip, remaining PSUM copies
  gpsimd: SBUF-only rope combine ops, big memsets
"""

import math
import os

import numpy as np
import ml_dtypes

import concourse.bass as bass
import concourse.mybir as mybir
import concourse.tile as _tile_mod
from concourse.tile import TileContext
from concourse.vector_clock import ScopedClock, VectorClock
import bass_rust as _bass_rust
from concourse.bass_utils import run_bass_kernel_spmd

_N_PROCS = _bass_rust.N_PROCS
BF = ml_dtypes.bfloat16


def _split_drain_and_barrier(self, tick_clock, wait_clock):
    """Replacement for TileContext._drain_and_barrier: the stock version puts
    the whole global vector clock (up to 27 sem waits) on one Drain, which this
    walrus rejects ("Too many sync wait commands").  Emit one Drain per
    outstanding processor instead."""
    gc = tick_clock.global_clock
    procs = [p for p in range(_N_PROCS) if gc[p] > 0]
    for p in procs:
        vc = VectorClock([gc[q] if q == p else 0 for q in range(_N_PROCS)])
        d = self.nc.sync.drain()
        wait_clock.add_sem_waits(d.ins, ScopedClock({None: vc}))
    self.nc.all_engine_barrier()
    popped = self.nc._tile_sem_poison_stack.pop()
    assert popped is self._sem_poison
    # single-execution NEFF: skip the semaphore-clearing gpsimd DMAs and the
    # second all-engine barrier (state does not persist past this run)


_tile_mod.TileContext._drain_and_barrier = _split_drain_and_barrier

# ---------------------------------------------------------------------------
# This walrus build allows only ONE sync-wait per instruction ("Too many sync
# wait commands").  Post-process the BIR JSON: excess waits are hoisted onto
# same-engine NoOp carriers inserted immediately before the instruction (same
# program point on the engine's sequential stream -> semantics unchanged).
# NoOp (unlike Drain) does not flush the engine pipeline: ~13ns vs ~500ns.
# ---------------------------------------------------------------------------
_orig_to_json_bytes = bass.Bass.to_json_bytes
_WAIT_LIMITS = {"Drain": 1, "DMACopy": 1}
_DEF_WAIT_LIMIT = 1


def _to_json_split_waits(self, *a, **kw):
    import json as _json
    data = _json.loads(_orig_to_json_bytes(self, *a, **kw))
    nid = 0
    for f in data.get("functions", []):
        for bb in f.get("blocks", []):
            out = []
            for inst in bb.get("instructions", []):
                si = inst.get("sync_info")
                if isinstance(si, dict):
                    w = si.get("on_wait")
                    if isinstance(w, list):
                        k = _WAIT_LIMITS.get(inst.get("opcode"), _DEF_WAIT_LIMIT)
                        if len(w) > k:
                            extra, keep = w[:-k], w[-k:]
                            for wt in extra:
                                out.append({
                                    "debug": inst.get("debug"),
                                    "engine": inst["engine"],
                                    "ins": [], "outs": [],
                                    "name": f"wsplit-{nid}",
                                    "opcode": "NoOp",
                                    "sync_info": {"on_update": [],
                                                  "on_wait": [wt]},
                                })
                                nid += 1
                            si["on_wait"] = keep
                out.append(inst)
            bb["instructions"] = out
    return _json.dumps(data).encode()


bass.Bass.to_json_bytes = _to_json_split_waits

B, T, E = 2, 2048, 1024
H, DH = 16, 64
DKV = 256
DR = 32
HL = 4              # heads per core
NG = 4              # head groups
SCALE = 1.0 / math.sqrt(DH + DR)
TG = 512            # query-group width
KC = 128            # key-chunk width
NTG = T // TG       # 4
NKC = T // KC       # 16
EC = E // 128       # 8  e-chunks
CC = DKV // 128     # 2  latent chunks

F32 = mybir.dt.float32
BF16 = mybir.dt.bfloat16
AF = mybir.ActivationFunctionType
ALU = mybir.AluOpType
AX = mybir.AxisListType

_CACHE = {}

_ROT16 = [(i + 16) % 32 for i in range(32)]
_IDENT = list(range(32))


def _build_program():
    nc = bass.Bass()

    xT = nc.declare_dram_parameter("xT", [E, T], BF16, isOutput=False)
    wq = nc.declare_dram_parameter("wq", [E, HL * 128], BF16, isOutput=False)
    wkvd = nc.declare_dram_parameter("wkvd", [E, DKV], BF16, isOutput=False)
    wku = nc.declare_dram_parameter("wku", [DKV, HL * DH], BF16, isOutput=False)
    wvu = nc.declare_dram_parameter("wvu", [DKV, HL * DH], BF16, isOutput=False)
    wo = nc.declare_dram_parameter("wo", [HL * DH, E], BF16, isOutput=False)
    cosq = nc.declare_dram_parameter("cosq", [128, T], BF16, isOutput=False)
    sinq = nc.declare_dram_parameter("sinq", [128, T], BF16, isOutput=False)
    cbc = nc.declare_dram_parameter("cbc", [32, HL * 16], F32, isOutput=False)
    masks = nc.declare_dram_parameter("masks", [128, 4 * TG], BF16, isOutput=False)
    ident = nc.declare_dram_parameter("ident", [128, 128], BF16, isOutput=False)
    out = nc.declare_dram_parameter("out", [T, E], BF16, isOutput=True)

    xTr = xT.rearrange("(c p) t -> p c t", p=128)

    with TileContext(nc) as tc:
        from contextlib import ExitStack

        with ExitStack() as ctx:
            singles = ctx.enter_context(tc.tile_pool(name="singles", bufs=1))
            pool = ctx.enter_context(tc.tile_pool(name="pool", bufs=2))
            psp = ctx.enter_context(tc.tile_pool(name="psp", bufs=1, space="PSUM"))

            # -------- weights + tables (bf16 from host), first-use order ---
            # per-ec chunked loads so the first projection matmul starts early
            wq_sb = singles.tile([128, EC, HL * 128], BF16)
            wqr_ = wq.rearrange("(c p) f -> p c f", p=128)
            for ec_ in range(EC):
                nc.sync.dma_start(out=wq_sb[:, ec_, :], in_=wqr_[:, ec_, :])

            xt_tiles = {}

            def load_xt(tg):
                ts_ = slice(tg * TG, (tg + 1) * TG)
                for ec_ in range(EC):
                    t_ = pool.tile([128, TG], BF16, name=f"xt{tg}_{ec_}",
                                   tag="xt", bufs=16)
                    nc.sync.dma_start(out=t_, in_=xTr[:, ec_, ts_])
                    xt_tiles[(tg, ec_)] = t_

            load_xt(0)
            wkvd_sb = singles.tile([128, EC, DKV], BF16)
            nc.sync.dma_start(
                out=wkvd_sb, in_=wkvd.rearrange("(c p) f -> p c f", p=128))
            wku_sb = singles.tile([128, CC, HL * DH], BF16)
            nc.sync.dma_start(out=wku_sb, in_=wku.rearrange("(c p) f -> p c f", p=128))
            wvu_sb = singles.tile([128, CC, HL * DH], BF16)
            nc.sync.dma_start(out=wvu_sb, in_=wvu.rearrange("(c p) f -> p c f", p=128))
            cos_sb = singles.tile([128, T], BF16)
            nc.sync.dma_start(out=cos_sb, in_=cosq[:, :])
            sin_sb = singles.tile([128, T], BF16)
            nc.sync.dma_start(out=sin_sb, in_=sinq[:, :])
            masks_sb = singles.tile([128, 4, TG], BF16)
            nc.sync.dma_start(
                out=masks_sb, in_=masks.rearrange("p (j y) -> p j y", j=4))
            ident_sb = singles.tile([128, 128], BF16)
            nc.sync.dma_start(out=ident_sb, in_=ident[:, :])
            cbc_sb = singles.tile([32, HL * 16], F32)
            nc.sync.dma_start(out=cbc_sb, in_=cbc[:, :])
            load_xt(1)
            wo_sb = singles.tile([128, 2, E], BF16)
            nc.sync.dma_start(out=wo_sb, in_=wo.rearrange("(c p) e -> p c e", p=128))

            ones1 = singles.tile([1, DH], BF16)
            nc.vector.memset(ones1, 1.0)
            dext = singles.tile([32, HL * TG], F32)
            nc.gpsimd.memset(dext, 1.0)
            rstage = singles.tile([32, HL * TG], BF16)
            nc.gpsimd.memset(rstage, 1.0)

            # ---------------- persistent activation tiles ----------------
            latT_sb = singles.tile([128, CC, T], BF16)
            qT = [singles.tile([96, T], BF16, name=f"qT{h}") for h in range(HL)]
            kT = [singles.tile([96, T], BF16, name=f"kT{h}") for h in range(HL)]
            v_sb = singles.tile([128, NKC, HL, DH + 1], BF16)
            nc.gpsimd.memset(v_sb, 1.0)
            yraw_sb = singles.tile([DH, HL, T], BF16)
            yT_sb = singles.tile([128, 2, T], BF16)

            norm_queue = []

            def drain_norms(n, keep=0):
                for _ in range(n):
                    if len(norm_queue) > keep:
                        norm_queue.pop(0)()

            def emit_norm(h, qg, rback_ap):
                qs = slice(qg * TG, (qg + 1) * TG)
                bcp = psp.tile([DH, TG], F32, name="bc", tag="B", bufs=2)
                nc.tensor.matmul(bcp, ones1, rback_ap)
                nc.vector.tensor_mul(
                    yT_sb[(h % 2) * DH:(h % 2 + 1) * DH, h // 2, qs],
                    yraw_sb[0:DH, h, qs], bcp)

            def emit_outproj(qg):
                for tt in range(4 * qg, 4 * qg + 4):
                    op = psp.tile([128, E], F32, name="op", tag="S", bufs=2)
                    for eg in range(2):
                        for fc in range(2):
                            nc.tensor.matmul(
                                op[:, eg * TG:(eg + 1) * TG],
                                yT_sb[:, fc, tt * 128:(tt + 1) * 128],
                                wo_sb[:, fc, eg * TG:(eg + 1) * TG],
                                start=(fc == 0), stop=(fc == 1))
                    ost = pool.tile([128, E], BF16, name="ost", tag="ost", bufs=2)
                    if tt % 2 == 0:
                        nc.vector.tensor_copy(ost, op)
                    else:
                        nc.scalar.copy(ost, op)
                    nc.sync.dma_start(
                        out=out[tt * 128:(tt + 1) * 128, :], in_=ost)

            def make_proj_fillers(tg):
                """Projection work for query-group tg as ~10 small closures,
                emitted between attention kc-pairs of the previous group so
                the tensor queue always has scores pending for the scalar
                exp pipeline."""
                ts = slice(tg * TG, (tg + 1) * TG)
                st = {}

                def mm8(tile_name, blk, hv):
                    xts = [xt_tiles[(tg, ec)] for ec in range(EC)]
                    if tile_name not in st:
                        st[tile_name] = psp.tile(
                            [128, 2 * TG], F32, name=tile_name, tag="S", bufs=2)
                    sp = st[tile_name]
                    fs = slice(blk * 128, (blk + 1) * 128)
                    for ec in range(EC):
                        nc.tensor.matmul(
                            sp[:, hv * TG:(hv + 1) * TG],
                            wq_sb[:, ec, fs] if tile_name != "lat"
                            else wkvd_sb[:, ec, blk * 128:(blk + 1) * 128],
                            xts[ec],
                            start=(ec == 0), stop=(ec == EC - 1))

                def f0():
                    if tg + 2 < NTG:
                        load_xt(tg + 2)
                    mm8("qc", 0, 0)

                def f1():
                    mm8("qc", 1, 1)
                    qcp = st["qc"]
                    nc.vector.tensor_copy(qT[0][0:DH, ts], qcp[0:DH, 0:TG])
                    nc.scalar.copy(qT[1][0:DH, ts], qcp[DH:128, 0:TG])
                    nc.vector.tensor_copy(qT[2][0:DH, ts], qcp[0:DH, TG:2 * TG])
                    nc.scalar.copy(qT[3][0:DH, ts], qcp[DH:128, TG:2 * TG])

                def f2():
                    mm8("qr", 2, 0)

                def f3():
                    mm8("qr", 3, 1)
                    qrp = st["qr"]
                    # q rope, 4 heads batched; SBUF-only combines on gpsimd
                    sw = pool.tile([128, TG], F32, name="sw", tag="qsw", bufs=2)
                    nc.vector.stream_shuffle(sw, qrp[:, 0:TG], _ROT16)
                    t1 = pool.tile([128, TG], BF16, name="t1", tag="t1", bufs=2)
                    nc.vector.tensor_mul(t1, qrp[:, 0:TG], cos_sb[:, ts])
                    t2 = pool.tile([128, TG], BF16, name="t2", tag="t2", bufs=2)
                    nc.gpsimd.tensor_mul(t2, sw, sin_sb[:, ts])
                    rq = pool.tile([128, TG], BF16, name="rq", tag="rq", bufs=2)
                    nc.gpsimd.tensor_add(rq, t1, t2)
                    for h in range(HL):
                        nc.scalar.copy(qT[h][DH:96, ts],
                                       rq[h * DR:(h + 1) * DR, :])
                    # k rope (rows 0:32 of the second half)
                    sw2 = pool.tile([DR, TG], F32, name="sw2", tag="sw2", bufs=2)
                    nc.vector.stream_shuffle(sw2, qrp[0:DR, TG:2 * TG], _ROT16)
                    k1 = pool.tile([DR, TG], BF16, name="k1", tag="k1", bufs=2)
                    nc.vector.tensor_mul(k1, qrp[0:DR, TG:2 * TG],
                                         cos_sb[0:DR, ts])
                    k2 = pool.tile([DR, TG], BF16, name="k2", tag="k2", bufs=2)
                    nc.gpsimd.tensor_mul(k2, sw2, sin_sb[0:DR, ts])
                    nc.vector.tensor_add(kT[0][DH:96, ts], k1, k2)
                    for h in range(1, HL):
                        nc.scalar.copy(kT[h][DH:96, ts], kT[0][DH:96, ts])

                def f4():
                    mm8("lat", 0, 0)

                def f5():
                    mm8("lat", 1, 1)
                    lp = st["lat"]
                    nc.vector.tensor_copy(latT_sb[:, 0, ts], lp[:, 0:TG])
                    nc.vector.tensor_copy(latT_sb[:, 1, ts], lp[:, TG:2 * TG])

                def kc_f(hp):
                    def f():
                        mp = psp.tile([128, TG], F32, name="kc", tag="M", bufs=2)
                        for cc in range(CC):
                            nc.tensor.matmul(
                                mp, wku_sb[:, cc, hp * 128:(hp + 1) * 128],
                                latT_sb[:, cc, ts],
                                start=(cc == 0), stop=(cc == CC - 1))
                        nc.vector.tensor_copy(kT[2 * hp][0:DH, ts], mp[0:DH, :])
                        nc.scalar.copy(kT[2 * hp + 1][0:DH, ts], mp[DH:128, :])
                    return f

                def v_f(tt0):
                    def f():
                        for tt in (tt0, tt0 + 1):
                            bp = psp.tile([128, HL * DH], F32, name="v",
                                          tag="B", bufs=2)
                            for cc in range(CC):
                                nc.tensor.matmul(
                                    bp, latT_sb[:, cc,
                                                tt * 128:(tt + 1) * 128],
                                    wvu_sb[:, cc, :],
                                    start=(cc == 0), stop=(cc == CC - 1))
                            nc.vector.tensor_copy(v_sb[:, tt, :, 0:DH], bp)
                    return f

                return [f0, f1, f2, f3, f4, f5,
                        kc_f(0), kc_f(1), v_f(4 * tg), v_f(4 * tg + 2)]

            def epilogue_half(qg, hf, comb, nh=2):
                """emax + denominator + reciprocal for heads h0..h0+nh."""
                h0 = 2 * hf if nh == 2 else hf
                cs = slice(h0 * TG, (h0 + nh) * TG)
                bs = slice(h0 * 16, (h0 + nh) * 16)
                W = nh * TG
                combT = pool.tile([128, W], BF16, name="combT", tag=f"combT{W}",
                                  bufs=1)
                nc.vector.transpose(
                    combT, comb.rearrange("p h y -> p (h y)")[:, cs])
                red = pool.tile([128, W // 32], F32, name="red", tag=f"red{W}", bufs=2)
                nc.vector.reduce_max(
                    red, combT.rearrange("p (b j) -> p b j", j=32), axis=AX.X)
                s1 = pool.tile([32, W // 32], F32, name="s1", tag=f"s1{W}", bufs=2)
                nc.vector.stream_shuffle(s1, red[32:64, :], _IDENT)
                s2 = pool.tile([32, W // 32], F32, name="s2", tag=f"s2{W}", bufs=2)
                nc.vector.stream_shuffle(s2, red[64:96, :], _IDENT)
                s3 = pool.tile([32, W // 32], F32, name="s3", tag=f"s3{W}", bufs=2)
                nc.vector.stream_shuffle(s3, red[96:128, :], _IDENT)
                nc.vector.tensor_max(s1, red[0:32, :], s1)
                nc.vector.tensor_max(s2, s2, s3)
                emfP = pool.tile([32, W // 32], F32, name="emf", tag=f"emf{W}", bufs=2)
                nc.vector.tensor_max(emfP, s1, s2)
                # d = dsum + C_h*emax (perm layout); C broadcast from host tile
                nc.vector.tensor_mul(emfP, emfP, cbc_sb[:, bs])
                dTt = pool.tile([32, W], F32, name="dTt", tag=f"dTt{W}", bufs=1)
                nc.vector.transpose(dTt, dext[:, cs])
                dP = pool.tile([32, W // 32], F32, name="dP", tag=f"dP{W}", bufs=2)
                nc.vector.tensor_add(
                    dP, emfP,
                    dTt.rearrange("p (b j) -> p b j", j=32)[:, :, 0])
                with nc.allow_low_precision(reason="r in bf16; tol 2e-2"):
                    nc.vector.reciprocal(
                        rstage[:, cs].rearrange("p (b j) -> p b j", j=32)
                        [:, :, 0], dP)
                rback = pool.tile([32, W], BF16, name="rbk", tag=f"rbk{W}", bufs=2)
                nc.vector.transpose(rback, rstage[:, cs])
                for hh in range(h0, h0 + nh):
                    norm_queue.append(
                        (lambda h_=hh, qq=qg,
                         r_=rback[0:1, (hh - h0) * TG:(hh - h0 + 1) * TG]:
                         emit_norm(h_, qq, r_)))

            # =======================================================
            for f in make_proj_fillers(0):
                f()
            for tg in range(NTG):
                qg = tg
                qs = slice(qg * TG, (qg + 1) * TG)
                nkc = 4 * qg + 4
                npair = nkc // 2
                diag = 4 * qg
                fillers = (make_proj_fillers(tg + 1)
                           if tg + 1 < NTG else [])
                fillers = list(fillers)

                def pop_filler():
                    if fillers:
                        fillers.pop(0)()

                # per-qg comb, all 4 heads side by side
                comb = pool.tile([128, HL, TG], BF16, name="comb", tag="comb",
                                 bufs=2)
                for h in range(HL):
                    yps = psp.tile([DH + 1, TG], F32, name="py", tag="M", bufs=2)

                    def emit_scores(pr):
                        kca, kcb = 2 * pr, 2 * pr + 1
                        sp = psp.tile([128, 2 * TG], F32, name="ps", tag="S",
                                      bufs=3)
                        for hv, kc in ((0, kca), (1, kcb)):
                            half = sp[:, hv * TG:(hv + 1) * TG]
                            j = kc - diag
                            nc.tensor.matmul(
                                half, kT[h][:, kc * KC:(kc + 1) * KC],
                                qT[h][:, qs],
                                start=True, stop=(j < 0))
                            if j >= 0:
                                # additive causal mask via ident matmul
                                w = 128 * j + 128
                                nc.tensor.matmul(
                                    sp[:, hv * TG:hv * TG + w],
                                    ident_sb, masks_sb[:, j, 0:w],
                                    start=False, stop=True)
                        pt = pool.tile([128, 2 * TG], BF16, name="pt", tag="pt",
                                       bufs=4)
                        nc.scalar.activation(pt, sp, AF.Exp)
                        return pt

                    def emit_pv(pr, pt):
                        if pr == 0:
                            nc.vector.tensor_max(
                                comb[:, h, :], pt[:, 0:TG], pt[:, TG:2 * TG])
                        else:
                            nc.vector.tensor_max(
                                comb[:, h, :], comb[:, h, :], pt[:, 0:TG])
                            nc.vector.tensor_max(
                                comb[:, h, :], comb[:, h, :], pt[:, TG:2 * TG])
                        nc.tensor.matmul(
                            yps, v_sb[:, 2 * pr, h, :], pt[:, 0:TG],
                            start=(pr == 0), stop=False)
                        nc.tensor.matmul(
                            yps, v_sb[:, 2 * pr + 1, h, :], pt[:, TG:2 * TG],
                            start=False, stop=(pr == npair - 1))

                    # software-pipeline scores one pair ahead of PV; proj
                    # fillers for the next query-group slot in between pairs
                    prev = emit_scores(0)
                    for pr in range(1, npair):
                        cur = emit_scores(pr)
                        emit_pv(pr - 1, prev)
                        pop_filler()
                        prev = cur
                    emit_pv(npair - 1, prev)
                    pop_filler()
                    nc.vector.tensor_copy(yraw_sb[:, h, qs], yps[0:DH, :])
                    # dsum (f32) into the h-major d row
                    nc.scalar.copy(dext[0:1, h * TG:(h + 1) * TG],
                                   yps[DH:DH + 1, :])
                    if h == 1:
                        epilogue_half(qg, 0, comb)
                    if tg == NTG - 1 and h >= 2:
                        epilogue_half(qg, h, comb, nh=1)
                    drain_norms(2 if tg == NTG - 1 else 1,
                                keep=0 if tg == NTG - 1 else 2)
                for f in fillers:
                    f()
                if tg < NTG - 1:
                    epilogue_half(qg, 1, comb)
                norm_queue.append(lambda qq=qg: emit_outproj(qq))
                drain_norms(1, keep=2)
            # tail
            drain_norms(len(norm_queue))

    return nc


def _masks():
    """Additive causal masks: -1e4 on masked (k > q) cells, 0 elsewhere."""
    x = np.arange(128)[:, None]
    y = np.arange(TG)[None, :]
    ms = [np.where(x - y + 128 * j <= 0, 0.0, -1e4).astype(np.float32)
          for j in range(4)]
    return np.concatenate(ms, axis=1)  # [128, 4*TG]


def _rope_tables():
    half = DR // 2
    inv = 1.0 / (10000.0 ** (np.arange(half, dtype=np.float64) / half))
    ang = np.arange(T, dtype=np.float64)[:, None] * inv[None, :]  # (T, half)
    cos = np.cos(ang).T  # (half, T)
    sin = np.sin(ang).T
    cosk = np.concatenate([cos, cos], axis=0)                 # (32, T)
    sink = np.concatenate([-sin, sin], axis=0)
    cosq = np.tile(cosk, (HL, 1))                             # (128, T)
    sinq = np.tile(sink, (HL, 1))
    return cosq.astype(np.float32), sinq.astype(np.float32)


def _bf(x):
    return np.ascontiguousarray(np.asarray(x, dtype=np.float32).astype(BF))


def kernel(x, Wq, Wqr, Wkr, Wkvd, Wku, Wvu, Wo, lobo_log):
    x = np.asarray(x, dtype=np.float32)
    Wq = np.asarray(Wq, dtype=np.float32)
    Wqr = np.asarray(Wqr, dtype=np.float32)
    Wkr = np.asarray(Wkr, dtype=np.float32)
    Wkvd = np.asarray(Wkvd, dtype=np.float32)
    Wku = np.asarray(Wku, dtype=np.float32)
    Wvu = np.asarray(Wvu, dtype=np.float32)
    Wo = np.asarray(Wo, dtype=np.float32)
    lobo_log = np.asarray(lobo_log, dtype=np.float32)

    if "nc" not in _CACHE:
        _CACHE["nc"] = _build_program()
    nc = _CACHE["nc"]

    cosq, sinq = _rope_tables()
    msk = _masks()
    in_maps = []
    for core in range(8):
        b, g = core // NG, core % NG
        hs = slice(g * HL * DH, (g + 1) * HL * DH)
        # packed q-projection weight blocks (128 cols each):
        #   0: [Wq_h0|Wq_h1]*S   1: [Wq_h2|Wq_h3]*S
        #   2: [Wqr_h0..h3]*S    3: [Wkr | zeros]
        wq_pack = np.zeros((E, HL * 128), dtype=np.float32)
        for h in range(HL):
            gh = g * HL + h
            blk, off = h // 2, (h % 2) * DH
            wq_pack[:, blk * 128 + off:blk * 128 + off + DH] = (
                Wq[:, gh * DH:(gh + 1) * DH] * SCALE)
            wq_pack[:, 256 + h * DR:256 + (h + 1) * DR] = (
                Wqr[:, gh * DR:(gh + 1) * DR] * SCALE)
        wq_pack[:, 384:384 + DR] = Wkr
        # C_h broadcast tile matching the (j, h*16+b) permuted layout
        cv = np.exp(lobo_log[g * HL:(g + 1) * HL])
        cbcv = np.broadcast_to(np.repeat(cv, 16)[None, :], (32, HL * 16))
        in_maps.append({
            "xT": _bf(x[b].T),
            "wq": _bf(wq_pack),
            "wkvd": _bf(Wkvd),
            "wku": _bf(Wku[:, hs]),
            "wvu": _bf(Wvu[:, hs]),
            "wo": _bf(Wo[hs, :]),
            "cosq": _bf(cosq), "sinq": _bf(sinq),
            "cbc": np.ascontiguousarray(cbcv, dtype=np.float32),
            "masks": _bf(msk),
            "ident": _bf(np.eye(128, dtype=np.float32)),
        })

    trace = bool(os.environ.get("BASS_TRACE_KERNEL"))
    bkr = run_bass_kernel_spmd(
        nc, in_maps, core_ids=list(range(8)), trace=trace)
    if trace:
        print(f"HW exec time: {bkr.exec_time_ns} ns")
        if bkr.instructions_and_trace is not None:
            print("trace:", bkr.instructions_and_trace[1])
        _CACHE["last_result"] = bkr
    res = bkr.results
    out = np.zeros((B, T, E), dtype=np.float32)
    for core in range(8):
        out[core // NG] += res[core]["out"].astype(np.float32)
    return out
